# revision 17
# baseline (speedup 1.0000x reference)
"""CrossModalityAttention Trainium2 kernel (v2: fp8 DoubleRow projections).

Full inputs -> full output; internally shards batch B=8192 across 8 NeuronCores
(pure data parallel). Per core: 1024 samples x K=8 modalities = 8192 tokens of
D=1024.

Device strategy (per core):
  - All four DxD projections run in fp8-e4m3 with perf_mode=DoubleRow (2
    fp8 MACs/cell/cycle, contraction chunk pairs packed in the AP's dim1).
    Power-of-2 quantization scales: x*32, wq/wv/wo*4096, (wk/sqrt(128))*32768,
    attention-output*32. Descales fold into the ACT bias stage (Q/K/V) or the
    2^17-prescaled residual XB (output proj), whose scale LayerNorm absorbs
    exactly via eps' = eps*2^34 (LN is scale-invariant).
  - Scores per 128-token group (16 samples x K=8) stay bf16:
    ST[(s,k),(s',q)] = Kh^T Qh via matmul(lhsT=kt, rhs=qt); prior/mask table
    adds rel_prior on the block diagonal and -30 off it; exp() -> bf16 probs.
  - Transpose-free tail: OT[d,q] = matmul(lhsT=V[k,d] bf16, rhs=P[k,q] bf16)
    gives the output projection's lhsT directly (no PE transposes). The
    softmax denominator reaches all 128 partitions via one rank-1 PE matmul
    ZB = (1/32 ones)^T @ P, then DVE reciprocal + multiply quantize OT to
    fp8 (the 1/32 folds the fp8 scale for free).
  - LayerNorm: rstd = exp(-0.5*ln(var+eps')) keeps every ACT function in one
    table set.
"""

import math

import numpy as np

import concourse.bacc as bacc
import concourse.bass as bass
import concourse.mybir as mybir
import concourse.tile as tile
from concourse.bass_utils import run_bass_kernel_spmd

N_CORES = 8
B, K, D = 8192, 8, 1024
H, HD = 8, 128
BC = B // N_CORES            # samples per core
T = BC * K                   # tokens per core (8192)
TS = 1024                    # tokens per tile
NT = T // TS                 # tiles per core
GROUPS = TS // 128           # 128-token groups per tile
SPG = 128 // K               # samples per group (16)
LN_EPS = 1e-5
NEG = -30.0                  # large-negative mask for cross-sample scores

XS = 32.0                    # fp8 scale for x and attention output
WS = 4096.0                  # fp8 scale for wq/wv/wo
KS = 32768.0                 # fp8 scale for wk/sqrt(HD)
RS = float(2 ** 17)          # residual prescale (= XS*WS); LN absorbs it

F32 = mybir.dt.float32
BF16 = mybir.dt.bfloat16
FP8 = mybir.dt.float8e4
DR = mybir.MatmulPerfMode.DoubleRow

_CACHED = None  # compiled Bacc module, built once per process


def _build():
    nc = bacc.Bacc("TRN2", target_bir_lowering=False, debug=False, num_devices=1)

    xt8_d = nc.dram_tensor("XT8", [D, T], FP8, kind="ExternalInput").ap()
    xb_d = nc.dram_tensor("XB", [T, D], F32, kind="ExternalInput").ap()
    wq_d = nc.dram_tensor("WQ8", [D, D], FP8, kind="ExternalInput").ap()
    wk_d = nc.dram_tensor("WK8", [D, D], FP8, kind="ExternalInput").ap()
    wv_d = nc.dram_tensor("WV8", [D, D], FP8, kind="ExternalInput").ap()
    wo_d = nc.dram_tensor("WO8", [D, D], FP8, kind="ExternalInput").ap()
    bqk_d = nc.dram_tensor("BQK", [128, 2 * H], F32, kind="ExternalInput").ap()
    pm_d = nc.dram_tensor("PM", [128, 128], F32, kind="ExternalInput").ap()
    om_d = nc.dram_tensor("OM", [128, 128], BF16, kind="ExternalInput").ap()
    out_d = nc.dram_tensor("OUT", [T, D], F32, kind="ExternalOutput").ap()

    xt8_r = xt8_d.rearrange("(c p) t -> p c t", p=128)   # [128, 8, T]

    with tile.TileContext(nc) as tc:
        with (
            tc.tile_pool(name="wpool", bufs=1) as wpool,
            tc.tile_pool(name="consts", bufs=1) as consts,
            tc.tile_pool(name="xt8p", bufs=2) as xt8p,
            tc.tile_pool(name="qkp", bufs=2) as qkp,
            tc.tile_pool(name="vp", bufs=1) as vp,
            tc.tile_pool(name="ptp", bufs=2) as ptp,
            tc.tile_pool(name="rzbp", bufs=2) as rzbp,
            tc.tile_pool(name="ot8p", bufs=2) as ot8p,
            tc.tile_pool(name="xbp", bufs=GROUPS + 1) as xbp,
            tc.tile_pool(name="smalls", bufs=4) as smalls,
            # two independent 2-bank x 2-buf PSUM streams: projections
            # (psab/psv/yp) and attention (st/zb/ot) — 8 banks total, and the
            # next tile's QK never contends with this tile's attention chain
            tc.tile_pool(name="projps", bufs=2, space="PSUM") as projps,
            tc.tile_pool(name="attps", bufs=2, space="PSUM") as attps,
        ):
            # ---- constants / weights (resident) ----
            wq = wpool.tile([128, 8, D], FP8, tag="w_q")
            nc.sync.dma_start(wq[:], wq_d.rearrange("(c p) m -> p c m", p=128))
            wk = wpool.tile([128, 8, D], FP8, tag="w_k")
            nc.sync.dma_start(wk[:], wk_d.rearrange("(c p) m -> p c m", p=128))
            wv = wpool.tile([128, 8, D], FP8, tag="w_v")
            nc.sync.dma_start(wv[:], wv_d.rearrange("(c p) m -> p c m", p=128))
            wo = wpool.tile([128, 8, D], FP8, tag="w_o")
            nc.sync.dma_start(wo[:], wo_d.rearrange("(c p) m -> p c m", p=128))
            bqk = consts.tile([128, 2 * H], F32)
            nc.sync.dma_start(bqk[:], bqk_d)
            pm = consts.tile([128, 128], F32)
            nc.sync.dma_start(pm[:], pm_d)
            om = consts.tile([128, 128], BF16)
            nc.sync.dma_start(om[:], om_d)
            eps = consts.tile([128, 1], F32)
            nc.vector.memset(eps[:], LN_EPS * RS * RS)

            for t in range(NT):
                tok0 = t * TS
                xt8 = xt8p.tile([128, 8, TS], FP8)
                nc.sync.dma_start(xt8[:], xt8_r[:, :, tok0 : tok0 + TS])

                # ---- Q^T, K^T projections (fp8 DoubleRow): [d_head x tok]
                qt = qkp.tile([128, H, TS], BF16, tag="qt")
                kt = qkp.tile([128, H, TS], BF16, tag="kt")
                for wt, dst, bias_col0, dsc in (
                    (wq, qt, 0, 1.0 / RS),
                    (wk, kt, H, 1.0 / (KS * XS)),
                ):
                    for h in range(H):
                        psab = projps.tile([128, 1024], F32, tag="projps")
                        for c in range(4):
                            lw = wt[:, 2 * c : 2 * c + 2, h * HD : (h + 1) * HD]
                            nc.tensor.matmul(
                                psab[:, 0:512], lw, xt8[:, 2 * c : 2 * c + 2, 0:512],
                                start=(c == 0), stop=(c == 3), perf_mode=DR,
                            )
                            nc.tensor.matmul(
                                psab[:, 512:1024], lw,
                                xt8[:, 2 * c : 2 * c + 2, 512:1024],
                                start=(c == 0), stop=(c == 3), perf_mode=DR,
                            )
                        nc.scalar.activation(
                            dst[:, h, :], psab[:],
                            mybir.ActivationFunctionType.Identity,
                            bias=bqk[:, bias_col0 + h : bias_col0 + h + 1],
                            scale=dsc,
                        )

                # ---- V projection (fp8 DoubleRow), token-major bf16
                v = vp.tile([128, GROUPS, H, HD], BF16, tag="v")
                for sub in range(GROUPS):
                    psv = projps.tile([128, 1024], F32, tag="projps")
                    for c in range(4):
                        lx = xt8[:, 2 * c : 2 * c + 2, sub * 128 : (sub + 1) * 128]
                        nc.tensor.matmul(
                            psv[:, 0:512], lx, wv[:, 2 * c : 2 * c + 2, 0:512],
                            start=(c == 0), stop=(c == 3), perf_mode=DR,
                        )
                        nc.tensor.matmul(
                            psv[:, 512:1024], lx, wv[:, 2 * c : 2 * c + 2, 512:1024],
                            start=(c == 0), stop=(c == 3), perf_mode=DR,
                        )
                    nc.scalar.activation(
                        v[:, sub, :, :], psv.rearrange("p (a b) -> p a b", a=8),
                        mybir.ActivationFunctionType.Copy, scale=1.0 / RS,
                    )

                # ---- attention + output proj + residual + LN per 128-tok group
                mvt = smalls.tile([128, GROUPS, 2], F32, tag="mvt")
                xbs = []
                for g in range(GROUPS):
                    xb = xbp.tile([128, D], F32)
                    nc.sync.dma_start(
                        xb[:], xb_d[tok0 + g * 128 : tok0 + (g + 1) * 128, :]
                    )
                    xbs.append(xb)
                for g in range(GROUPS):
                    gsl = slice(g * 128, (g + 1) * 128)
                    st = attps.tile([128, H, 128], F32, tag="attps")
                    for h in range(H):
                        nc.tensor.matmul(st[:, h, :], kt[:, h, gsl], qt[:, h, gsl])
                    # add prior/mask (same [128,128] table per head), in place
                    nc.vector.tensor_tensor(
                        st[:], st[:],
                        pm[:, None, :].to_broadcast((128, H, 128)),
                        mybir.AluOpType.add,
                    )
                    pt = ptp.tile([128, H, 128], BF16)
                    nc.scalar.activation(
                        pt[:], st[:], mybir.ActivationFunctionType.Exp
                    )
                    # denominator, broadcast to all partitions: ZB = (1/32)^T P
                    zb = attps.tile([128, 1024], F32, tag="attps")
                    nc.tensor.matmul(zb[:, 0:512], om[:], pt[:, 0:4, :])
                    nc.tensor.matmul(zb[:, 512:1024], om[:], pt[:, 4:8, :])
                    rzb = rzbp.tile([128, H, 128], F32)
                    nc.vector.reciprocal_approx_fast(
                        rzb[:], zb.rearrange("p (a b) -> p a b", a=8)
                    )
                    # OT[d, q] = V^T P per head (lhsT=v), then *32/Z -> fp8
                    ot = attps.tile([128, H, 128], F32, tag="attps")
                    for h in range(H):
                        nc.tensor.matmul(ot[:, h, :], v[:, g, h, :], pt[:, h, :])
                    ot8 = ot8p.tile([128, H, 128], FP8)
                    nc.vector.tensor_tensor(
                        ot8[:], ot[:], rzb[:], mybir.AluOpType.mult
                    )

                    xb = xbs[g]
                    yp = projps.tile([128, 1024], F32, tag="projps")
                    for c in range(4):
                        lo = ot8[:, 2 * c : 2 * c + 2, :]
                        nc.tensor.matmul(
                            yp[:, 0:512], lo, wo[:, 2 * c : 2 * c + 2, 0:512],
                            start=(c == 0), stop=(c == 3), perf_mode=DR,
                        )
                        nc.tensor.matmul(
                            yp[:, 512:1024], lo, wo[:, 2 * c : 2 * c + 2, 512:1024],
                            start=(c == 0), stop=(c == 3), perf_mode=DR,
                        )
                    nc.vector.tensor_tensor(
                        xb[:], xb[:], yp[:], mybir.AluOpType.add
                    )
                    stats = smalls.tile([128, 2, 6], F32, tag="stats")
                    for sg in range(2):
                        nc.vector.bn_stats(
                            stats[:, sg, :], xb[:, sg * 512 : (sg + 1) * 512]
                        )
                    nc.vector.bn_aggr(mvt[:, g, :], stats[:])

                # rstd = exp(-0.5*ln(var+eps')) batched: table switches happen
                # per tile, not per group. Last tile splits in half to shorten
                # the drain after the final matmul.
                batches = ((0, 4), (4, 8)) if t == NT - 1 else ((0, GROUPS),)
                sdt = smalls.tile([128, GROUPS], F32, tag="sdt")
                for lo, hi in batches:
                    nc.scalar.activation(
                        sdt[:, lo:hi], mvt[:, lo:hi, 1],
                        mybir.ActivationFunctionType.Ln, bias=eps[:],
                    )
                    nc.scalar.activation(
                        sdt[:, lo:hi], sdt[:, lo:hi],
                        mybir.ActivationFunctionType.Exp, scale=-0.5,
                    )
                    for g in range(lo, hi):
                        xb = xbs[g]
                        nc.vector.tensor_scalar(
                            out=xb[:],
                            in0=xb[:],
                            scalar1=mvt[:, g, 0:1],
                            scalar2=sdt[:, g : g + 1],
                            op0=mybir.AluOpType.subtract,
                            op1=mybir.AluOpType.mult,
                        )
                        nc.sync.dma_start(
                            out_d[tok0 + g * 128 : tok0 + (g + 1) * 128, :], xb[:]
                        )

    nc.compile()
    return nc


def _get_nc():
    global _CACHED
    if _CACHED is None:
        _CACHED = _build()
    return _CACHED


def _reference_numpy(modality_encodings, selection_mask, wq, bq, wk, bk, wv, bv,
                     wo, bo, rel_prior, ln_gamma, ln_beta):
    """Slow fallback, exact port of the reference (used only if inputs fall
    outside the fast path's assumptions: non-trivial mask)."""
    x = modality_encodings.astype(np.float32)
    Bn, Kn, Dn = x.shape
    Hd = Dn // H
    q = (x @ wq.T + bq).reshape(Bn, Kn, H, Hd).transpose(0, 2, 1, 3)
    k = (x @ wk.T + bk).reshape(Bn, Kn, H, Hd).transpose(0, 2, 1, 3)
    v = (x @ wv.T + bv).reshape(Bn, Kn, H, Hd).transpose(0, 2, 1, 3)
    scores = np.einsum("bhqd,bhkd->bhqk", q, k) / math.sqrt(Hd)
    scores = scores + rel_prior[None, None]
    mask2d = (selection_mask[:, :, None] * selection_mask[:, None, :]) > 0
    scores = np.where(mask2d[:, None], scores, -np.inf)
    scores = scores - scores.max(axis=-1, keepdims=True)
    e = np.exp(scores)
    attn = e / e.sum(axis=-1, keepdims=True)
    out = np.einsum("bhqk,bhkd->bhqd", attn, v)
    out = out.transpose(0, 2, 1, 3).reshape(Bn, Kn, Dn)
    out = out @ wo.T + bo
    res = x + out
    mu = res.mean(-1, keepdims=True)
    var = ((res - mu) ** 2).mean(-1, keepdims=True)
    return (res - mu) / np.sqrt(var + LN_EPS) * ln_gamma + ln_beta


def _q8(a, scale):
    import ml_dtypes

    return np.clip(a * scale, -240.0, 240.0).astype(ml_dtypes.float8_e4m3)


def _prep_in_maps(modality_encodings, wq, bq, wk, bk, wv, bv, wo, bo, rel_prior):
    import ml_dtypes

    s = 1.0 / math.sqrt(HD)
    wq8 = _q8(np.ascontiguousarray(wq.T), WS)
    wk8 = _q8(np.ascontiguousarray((wk * s).T), KS)
    wv8 = _q8(np.ascontiguousarray(wv.T), WS)
    wo8 = _q8(np.ascontiguousarray(wo.T), WS)
    bks = bk * s
    b_eff = (bo + wo @ bv).astype(np.float32)

    bqk = np.concatenate(
        [bq.reshape(H, HD).T, bks.reshape(H, HD).T], axis=1
    ).astype(np.float32)  # [128, 16]

    pmat = np.full((128, 128), NEG, dtype=np.float32)
    for sm in range(SPG):
        pmat[sm * K : (sm + 1) * K, sm * K : (sm + 1) * K] = rel_prior.T
    omat = np.full((128, 128), 1.0 / XS, dtype=ml_dtypes.bfloat16)

    x_flat = modality_encodings.reshape(B * K, D)
    in_maps = []
    for c in range(N_CORES):
        x_core = x_flat[c * T : (c + 1) * T]
        in_maps.append({
            "XT8": _q8(np.ascontiguousarray(x_core.T), XS),
            "XB": (x_core + b_eff) * RS,
            "WQ8": wq8, "WK8": wk8, "WV8": wv8, "WO8": wo8,
            "BQK": bqk, "PM": pmat, "OM": omat,
        })
    return in_maps


def run_device(inputs, trace=False):
    """Build in_maps from full inputs, run on 8 cores, return (full_out, results)."""
    in_maps = _prep_in_maps(
        inputs["modality_encodings"], inputs["wq"], inputs["bq"], inputs["wk"],
        inputs["bk"], inputs["wv"], inputs["bv"], inputs["wo"], inputs["bo"],
        inputs["rel_prior"],
    )
    nc = _get_nc()
    res = run_bass_kernel_spmd(nc, in_maps, core_ids=list(range(N_CORES)), trace=trace)
    out = np.concatenate(
        [res.results[c]["OUT"].reshape(BC, K, D) for c in range(N_CORES)], axis=0
    )
    return out, res


def kernel(**inputs) -> np.ndarray:
    inputs = {k: np.asarray(v) for k, v in inputs.items()}
    mask = inputs["selection_mask"]
    gamma = inputs["ln_gamma"]
    beta = inputs["ln_beta"]
    if not np.all(mask > 0):
        # general-mask fallback (never hit for the spec'd inputs: fill=ones)
        return _reference_numpy(**{k: inputs[k].astype(np.float32) for k in (
            "modality_encodings", "selection_mask", "wq", "bq", "wk", "bk",
            "wv", "bv", "wo", "bo", "rel_prior", "ln_gamma", "ln_beta")}
        ).astype(np.float32)

    out, _ = run_device(inputs, trace=False)
    # device kernel skips the (identity for spec'd inputs) LN affine params
    if not (np.all(gamma == 1.0) and np.all(beta == 0.0)):
        out = out * gamma + beta
    return out.astype(np.float32)


# revision 20
# speedup vs baseline: 1.0045x; 1.0045x over previous
"""CrossModalityAttention Trainium2 kernel (v2: fp8 DoubleRow projections).

Full inputs -> full output; internally shards batch B=8192 across 8 NeuronCores
(pure data parallel). Per core: 1024 samples x K=8 modalities = 8192 tokens of
D=1024.

Device strategy (per core):
  - All four DxD projections run in fp8-e4m3 with perf_mode=DoubleRow (2
    fp8 MACs/cell/cycle, contraction chunk pairs packed in the AP's dim1).
    Power-of-2 quantization scales: x*32, wq/wv/wo*4096, (wk/sqrt(128))*32768,
    attention-output*32. Descales fold into the ACT bias stage (Q/K/V) or the
    2^17-prescaled residual XB (output proj), whose scale LayerNorm absorbs
    exactly via eps' = eps*2^34 (LN is scale-invariant).
  - Scores per 128-token group (16 samples x K=8) stay bf16:
    ST[(s,k),(s',q)] = Kh^T Qh via matmul(lhsT=kt, rhs=qt); prior/mask table
    adds rel_prior on the block diagonal and -30 off it; exp() -> bf16 probs.
  - Transpose-free tail: OT[d,q] = matmul(lhsT=V[k,d] bf16, rhs=P[k,q] bf16)
    gives the output projection's lhsT directly (no PE transposes). The
    softmax denominator reaches all 128 partitions via one rank-1 PE matmul
    ZB = (1/32 ones)^T @ P, then DVE reciprocal + multiply quantize OT to
    fp8 (the 1/32 folds the fp8 scale for free).
  - LayerNorm: rstd = exp(-0.5*ln(var+eps')) keeps every ACT function in one
    table set.
"""

import math

import numpy as np

import concourse.bacc as bacc
import concourse.bass as bass
import concourse.mybir as mybir
import concourse.tile as tile
from concourse.bass_utils import run_bass_kernel_spmd

N_CORES = 8
B, K, D = 8192, 8, 1024
H, HD = 8, 128
BC = B // N_CORES            # samples per core
T = BC * K                   # tokens per core (8192)
TS = 1024                    # tokens per tile
NT = T // TS                 # tiles per core
GROUPS = TS // 128           # 128-token groups per tile
SPG = 128 // K               # samples per group (16)
LN_EPS = 1e-5
NEG = -30.0                  # large-negative mask for cross-sample scores

XS = 32.0                    # fp8 scale for x and attention output
WS = 4096.0                  # fp8 scale for wq/wv/wo
KS = 32768.0                 # fp8 scale for wk/sqrt(HD)
RS = float(2 ** 17)          # residual prescale (= XS*WS); LN absorbs it

F32 = mybir.dt.float32
BF16 = mybir.dt.bfloat16
FP8 = mybir.dt.float8e4
DR = mybir.MatmulPerfMode.DoubleRow

_CACHED = None  # compiled Bacc module, built once per process


def _build():
    nc = bacc.Bacc("TRN2", target_bir_lowering=False, debug=False, num_devices=1)

    xt8_d = nc.dram_tensor("XT8", [D, T], FP8, kind="ExternalInput").ap()
    xb_d = nc.dram_tensor("XB", [T, D], F32, kind="ExternalInput").ap()
    wq_d = nc.dram_tensor("WQ8", [D, D], FP8, kind="ExternalInput").ap()
    wk_d = nc.dram_tensor("WK8", [D, D], FP8, kind="ExternalInput").ap()
    wv_d = nc.dram_tensor("WV8", [D, D], FP8, kind="ExternalInput").ap()
    wo_d = nc.dram_tensor("WO8", [D, D], FP8, kind="ExternalInput").ap()
    bqk_d = nc.dram_tensor("BQK", [128, 2 * H], F32, kind="ExternalInput").ap()
    pm_d = nc.dram_tensor("PM", [128, 128], F32, kind="ExternalInput").ap()
    om_d = nc.dram_tensor("OM", [128, 128], BF16, kind="ExternalInput").ap()
    out_d = nc.dram_tensor("OUT", [T, D], F32, kind="ExternalOutput").ap()

    xt8_r = xt8_d.rearrange("(c p) t -> p c t", p=128)   # [128, 8, T]

    with tile.TileContext(nc) as tc:
        with (
            tc.tile_pool(name="wpool", bufs=1) as wpool,
            tc.tile_pool(name="consts", bufs=1) as consts,
            tc.tile_pool(name="xt8p", bufs=2) as xt8p,
            tc.tile_pool(name="qkp", bufs=2) as qkp,
            tc.tile_pool(name="vp", bufs=2) as vp,
            tc.tile_pool(name="ptp", bufs=2) as ptp,
            tc.tile_pool(name="rzbp", bufs=2) as rzbp,
            tc.tile_pool(name="ot8p", bufs=2) as ot8p,
            tc.tile_pool(name="xbp", bufs=GROUPS + 1) as xbp,
            tc.tile_pool(name="smalls", bufs=4) as smalls,
            # two independent 2-bank x 2-buf PSUM streams: projections
            # (psab/psv/yp) and attention (st/zb/ot) — 8 banks total, and the
            # next tile's QK never contends with this tile's attention chain
            tc.tile_pool(name="projps", bufs=2, space="PSUM") as projps,
            tc.tile_pool(name="attps", bufs=2, space="PSUM") as attps,
        ):
            # ---- constants / weights (resident) ----
            wq = wpool.tile([128, 8, D], FP8, tag="w_q")
            nc.sync.dma_start(wq[:], wq_d.rearrange("(c p) m -> p c m", p=128))
            wk = wpool.tile([128, 8, D], FP8, tag="w_k")
            nc.sync.dma_start(wk[:], wk_d.rearrange("(c p) m -> p c m", p=128))
            wv = wpool.tile([128, 8, D], FP8, tag="w_v")
            nc.sync.dma_start(wv[:], wv_d.rearrange("(c p) m -> p c m", p=128))
            wo = wpool.tile([128, 8, D], FP8, tag="w_o")
            nc.sync.dma_start(wo[:], wo_d.rearrange("(c p) m -> p c m", p=128))
            bqk = consts.tile([128, 2 * H], F32)
            nc.sync.dma_start(bqk[:], bqk_d)
            pm = consts.tile([128, 128], F32)
            nc.sync.dma_start(pm[:], pm_d)
            om = consts.tile([128, 128], BF16)
            nc.sync.dma_start(om[:], om_d)
            eps = consts.tile([128, 1], F32)
            nc.vector.memset(eps[:], LN_EPS * RS * RS)

            def start_tile(t):
                """Allocate tile t's SBUF tiles + input DMAs; return state."""
                tok0 = t * TS
                xt8 = xt8p.tile([128, 8, TS], FP8)
                nc.sync.dma_start(xt8[:], xt8_r[:, :, tok0 : tok0 + TS])
                s = {
                    "tok0": tok0,
                    "xt8": xt8,
                    "qt": qkp.tile([128, H, TS], BF16, tag="qt", name="qt"),
                    "kt": qkp.tile([128, H, TS], BF16, tag="kt", name="kt"),
                    "v": vp.tile([128, GROUPS, H, HD], BF16, tag="v", name="v"),
                    "mvt": smalls.tile([128, GROUPS, 2], F32, tag="mvt", name="mvt"),
                    "xbs": [],
                }
                for g in range(GROUPS):
                    xb = xbp.tile([128, D], F32)
                    nc.sync.dma_start(
                        xb[:], xb_d[tok0 + g * 128 : tok0 + (g + 1) * 128, :]
                    )
                    s["xbs"].append(xb)
                return s

            def emit_qk_unit(s, u):
                """One head of the Q or K projection (fp8 DoubleRow)."""
                proj, h = divmod(u, H)
                wt, dst, bias_col0, dsc = (
                    (wq, s["qt"], 0, 1.0 / RS),
                    (wk, s["kt"], H, 1.0 / (KS * XS)),
                )[proj]
                xt8 = s["xt8"]
                psab = projps.tile([128, 1024], F32, tag="projps")
                for c in range(4):
                    lw = wt[:, 2 * c : 2 * c + 2, h * HD : (h + 1) * HD]
                    nc.tensor.matmul(
                        psab[:, 0:512], lw, xt8[:, 2 * c : 2 * c + 2, 0:512],
                        start=(c == 0), stop=(c == 3), perf_mode=DR,
                    )
                    nc.tensor.matmul(
                        psab[:, 512:1024], lw, xt8[:, 2 * c : 2 * c + 2, 512:1024],
                        start=(c == 0), stop=(c == 3), perf_mode=DR,
                    )
                nc.scalar.activation(
                    dst[:, h, :], psab[:],
                    mybir.ActivationFunctionType.Identity,
                    bias=bqk[:, bias_col0 + h : bias_col0 + h + 1],
                    scale=dsc,
                )

            def emit_v_unit(s, sub):
                """One 128-token group of the V projection (fp8 DoubleRow)."""
                xt8 = s["xt8"]
                psv = projps.tile([128, 1024], F32, tag="projps")
                for c in range(4):
                    lx = xt8[:, 2 * c : 2 * c + 2, sub * 128 : (sub + 1) * 128]
                    nc.tensor.matmul(
                        psv[:, 0:512], lx, wv[:, 2 * c : 2 * c + 2, 0:512],
                        start=(c == 0), stop=(c == 3), perf_mode=DR,
                    )
                    nc.tensor.matmul(
                        psv[:, 512:1024], lx, wv[:, 2 * c : 2 * c + 2, 512:1024],
                        start=(c == 0), stop=(c == 3), perf_mode=DR,
                    )
                nc.scalar.activation(
                    s["v"][:, sub, :, :], psv.rearrange("p (a b) -> p a b", a=8),
                    mybir.ActivationFunctionType.Copy, scale=1.0 / RS,
                )

            def emit_unit(s, u):
                if u < 2 * H:
                    emit_qk_unit(s, u)
                else:
                    emit_v_unit(s, u - 2 * H)

            def emit_group(s, g):
                """Attention + output projection + residual + LN stats for one
                128-token (16-sample) group."""
                qt, kt, v = s["qt"], s["kt"], s["v"]
                gsl = slice(g * 128, (g + 1) * 128)
                st = attps.tile([128, H, 128], F32, tag="attps")
                for h in range(H):
                    nc.tensor.matmul(st[:, h, :], kt[:, h, gsl], qt[:, h, gsl])
                # add prior/mask (same [128,128] table per head), in place
                nc.vector.tensor_tensor(
                    st[:], st[:],
                    pm[:, None, :].to_broadcast((128, H, 128)),
                    mybir.AluOpType.add,
                )
                pt = ptp.tile([128, H, 128], BF16)
                nc.scalar.activation(pt[:], st[:], mybir.ActivationFunctionType.Exp)
                # denominator, broadcast to all partitions: ZB = (1/32)^T P
                zb = attps.tile([128, 1024], F32, tag="attps")
                nc.tensor.matmul(zb[:, 0:512], om[:], pt[:, 0:4, :])
                nc.tensor.matmul(zb[:, 512:1024], om[:], pt[:, 4:8, :])
                rzb = rzbp.tile([128, H, 128], F32)
                nc.vector.reciprocal_approx_fast(
                    rzb[:], zb.rearrange("p (a b) -> p a b", a=8)
                )
                # OT[d, q] = V^T P per head (lhsT=v), then *32/Z -> fp8
                ot = attps.tile([128, H, 128], F32, tag="attps")
                for h in range(H):
                    nc.tensor.matmul(ot[:, h, :], v[:, g, h, :], pt[:, h, :])
                ot8 = ot8p.tile([128, H, 128], FP8)
                nc.vector.tensor_tensor(ot8[:], ot[:], rzb[:], mybir.AluOpType.mult)

                xb = s["xbs"][g]
                yp = attps.tile([128, 1024], F32, tag="attps")
                for c in range(4):
                    lo = ot8[:, 2 * c : 2 * c + 2, :]
                    nc.tensor.matmul(
                        yp[:, 0:512], lo, wo[:, 2 * c : 2 * c + 2, 0:512],
                        start=(c == 0), stop=(c == 3), perf_mode=DR,
                    )
                    nc.tensor.matmul(
                        yp[:, 512:1024], lo, wo[:, 2 * c : 2 * c + 2, 512:1024],
                        start=(c == 0), stop=(c == 3), perf_mode=DR,
                    )
                nc.vector.tensor_tensor(xb[:], xb[:], yp[:], mybir.AluOpType.add)
                stats = smalls.tile([128, 2, 6], F32, tag="stats")
                for sg in range(2):
                    nc.vector.bn_stats(
                        stats[:, sg, :], xb[:, sg * 512 : (sg + 1) * 512]
                    )
                nc.vector.bn_aggr(s["mvt"][:, g, :], stats[:])

            def emit_ln_finalize(s, last):
                """rstd = exp(-0.5*ln(var+eps')) batched: table switches happen
                per tile, not per group. The last tile splits in half to
                shorten the drain after the final matmul."""
                tok0, mvt = s["tok0"], s["mvt"]
                batches = ((0, 4), (4, 8)) if last else ((0, GROUPS),)
                sdt = smalls.tile([128, GROUPS], F32, tag="sdt")
                for lo, hi in batches:
                    nc.scalar.activation(
                        sdt[:, lo:hi], mvt[:, lo:hi, 1],
                        mybir.ActivationFunctionType.Ln, bias=eps[:],
                    )
                    nc.scalar.activation(
                        sdt[:, lo:hi], sdt[:, lo:hi],
                        mybir.ActivationFunctionType.Exp, scale=-0.5,
                    )
                    for g in range(lo, hi):
                        xb = s["xbs"][g]
                        nc.vector.tensor_scalar(
                            out=xb[:],
                            in0=xb[:],
                            scalar1=mvt[:, g, 0:1],
                            scalar2=sdt[:, g : g + 1],
                            op0=mybir.AluOpType.subtract,
                            op1=mybir.AluOpType.mult,
                        )
                        nc.sync.dma_start(
                            out_d[tok0 + g * 128 : tok0 + (g + 1) * 128, :], xb[:]
                        )

            # Software pipeline: tile t's 24 projection units (PE-dense, no
            # deps on tile t-1) are interleaved 3-per-group with tile t-1's
            # attention groups, so they fill the PE bubbles left by each
            # group's exp -> reciprocal -> quantize dependency chain.
            prev = None
            for t in range(NT):
                s = start_tile(t)
                if prev is None:
                    for u in range(3 * GROUPS):
                        emit_unit(s, u)
                else:
                    for g in range(GROUPS):
                        for u in range(3 * g, 3 * (g + 1)):
                            emit_unit(s, u)
                        emit_group(prev, g)
                    emit_ln_finalize(prev, last=False)
                prev = s
            for g in range(GROUPS):
                emit_group(prev, g)
            emit_ln_finalize(prev, last=True)

    nc.compile()
    return nc


def _get_nc():
    global _CACHED
    if _CACHED is None:
        _CACHED = _build()
    return _CACHED


def _reference_numpy(modality_encodings, selection_mask, wq, bq, wk, bk, wv, bv,
                     wo, bo, rel_prior, ln_gamma, ln_beta):
    """Slow fallback, exact port of the reference (used only if inputs fall
    outside the fast path's assumptions: non-trivial mask)."""
    x = modality_encodings.astype(np.float32)
    Bn, Kn, Dn = x.shape
    Hd = Dn // H
    q = (x @ wq.T + bq).reshape(Bn, Kn, H, Hd).transpose(0, 2, 1, 3)
    k = (x @ wk.T + bk).reshape(Bn, Kn, H, Hd).transpose(0, 2, 1, 3)
    v = (x @ wv.T + bv).reshape(Bn, Kn, H, Hd).transpose(0, 2, 1, 3)
    scores = np.einsum("bhqd,bhkd->bhqk", q, k) / math.sqrt(Hd)
    scores = scores + rel_prior[None, None]
    mask2d = (selection_mask[:, :, None] * selection_mask[:, None, :]) > 0
    scores = np.where(mask2d[:, None], scores, -np.inf)
    scores = scores - scores.max(axis=-1, keepdims=True)
    e = np.exp(scores)
    attn = e / e.sum(axis=-1, keepdims=True)
    out = np.einsum("bhqk,bhkd->bhqd", attn, v)
    out = out.transpose(0, 2, 1, 3).reshape(Bn, Kn, Dn)
    out = out @ wo.T + bo
    res = x + out
    mu = res.mean(-1, keepdims=True)
    var = ((res - mu) ** 2).mean(-1, keepdims=True)
    return (res - mu) / np.sqrt(var + LN_EPS) * ln_gamma + ln_beta


def _q8(a, scale):
    import ml_dtypes

    return np.clip(a * scale, -240.0, 240.0).astype(ml_dtypes.float8_e4m3)


def _prep_in_maps(modality_encodings, wq, bq, wk, bk, wv, bv, wo, bo, rel_prior):
    import ml_dtypes

    s = 1.0 / math.sqrt(HD)
    wq8 = _q8(np.ascontiguousarray(wq.T), WS)
    wk8 = _q8(np.ascontiguousarray((wk * s).T), KS)
    wv8 = _q8(np.ascontiguousarray(wv.T), WS)
    wo8 = _q8(np.ascontiguousarray(wo.T), WS)
    bks = bk * s
    b_eff = (bo + wo @ bv).astype(np.float32)

    bqk = np.concatenate(
        [bq.reshape(H, HD).T, bks.reshape(H, HD).T], axis=1
    ).astype(np.float32)  # [128, 16]

    pmat = np.full((128, 128), NEG, dtype=np.float32)
    for sm in range(SPG):
        pmat[sm * K : (sm + 1) * K, sm * K : (sm + 1) * K] = rel_prior.T
    omat = np.full((128, 128), 1.0 / XS, dtype=ml_dtypes.bfloat16)

    x_flat = modality_encodings.reshape(B * K, D)
    in_maps = []
    for c in range(N_CORES):
        x_core = x_flat[c * T : (c + 1) * T]
        in_maps.append({
            "XT8": _q8(np.ascontiguousarray(x_core.T), XS),
            "XB": (x_core + b_eff) * RS,
            "WQ8": wq8, "WK8": wk8, "WV8": wv8, "WO8": wo8,
            "BQK": bqk, "PM": pmat, "OM": omat,
        })
    return in_maps


def run_device(inputs, trace=False):
    """Build in_maps from full inputs, run on 8 cores, return (full_out, results)."""
    in_maps = _prep_in_maps(
        inputs["modality_encodings"], inputs["wq"], inputs["bq"], inputs["wk"],
        inputs["bk"], inputs["wv"], inputs["bv"], inputs["wo"], inputs["bo"],
        inputs["rel_prior"],
    )
    nc = _get_nc()
    res = run_bass_kernel_spmd(nc, in_maps, core_ids=list(range(N_CORES)), trace=trace)
    out = np.concatenate(
        [res.results[c]["OUT"].reshape(BC, K, D) for c in range(N_CORES)], axis=0
    )
    return out, res


def kernel(**inputs) -> np.ndarray:
    inputs = {k: np.asarray(v) for k, v in inputs.items()}
    mask = inputs["selection_mask"]
    gamma = inputs["ln_gamma"]
    beta = inputs["ln_beta"]
    if not np.all(mask > 0):
        # general-mask fallback (never hit for the spec'd inputs: fill=ones)
        return _reference_numpy(**{k: inputs[k].astype(np.float32) for k in (
            "modality_encodings", "selection_mask", "wq", "bq", "wk", "bk",
            "wv", "bv", "wo", "bo", "rel_prior", "ln_gamma", "ln_beta")}
        ).astype(np.float32)

    out, _ = run_device(inputs, trace=False)
    # device kernel skips the (identity for spec'd inputs) LN affine params
    if not (np.all(gamma == 1.0) and np.all(beta == 0.0)):
        out = out * gamma + beta
    return out.astype(np.float32)


# revision 22
# speedup vs baseline: 1.0083x; 1.0037x over previous
"""CrossModalityAttention Trainium2 kernel (v2: fp8 DoubleRow projections).

Full inputs -> full output; internally shards batch B=8192 across 8 NeuronCores
(pure data parallel). Per core: 1024 samples x K=8 modalities = 8192 tokens of
D=1024.

Device strategy (per core):
  - All four DxD projections run in fp8-e4m3 with perf_mode=DoubleRow (2
    fp8 MACs/cell/cycle, contraction chunk pairs packed in the AP's dim1).
    Power-of-2 quantization scales: x*32, wq/wv/wo*4096, (wk/sqrt(128))*32768,
    attention-output*32. Descales fold into the ACT bias stage (Q/K/V) or the
    2^17-prescaled residual XB (output proj), whose scale LayerNorm absorbs
    exactly via eps' = eps*2^34 (LN is scale-invariant).
  - Scores per 128-token group (16 samples x K=8) stay bf16:
    ST[(s,k),(s',q)] = Kh^T Qh via matmul(lhsT=kt, rhs=qt); prior/mask table
    adds rel_prior on the block diagonal and -30 off it; exp() -> bf16 probs.
  - Transpose-free tail: OT[d,q] = matmul(lhsT=V[k,d] bf16, rhs=P[k,q] bf16)
    gives the output projection's lhsT directly (no PE transposes). The
    softmax denominator reaches all 128 partitions via one rank-1 PE matmul
    ZB = (1/32 ones)^T @ P, then DVE reciprocal + multiply quantize OT to
    fp8 (the 1/32 folds the fp8 scale for free).
  - LayerNorm: rstd = exp(-0.5*ln(var+eps')) keeps every ACT function in one
    table set.
"""

import math

import numpy as np

import concourse.bacc as bacc
import concourse.bass as bass
import concourse.mybir as mybir
import concourse.tile as tile
from concourse.bass_utils import run_bass_kernel_spmd

N_CORES = 8
B, K, D = 8192, 8, 1024
H, HD = 8, 128
BC = B // N_CORES            # samples per core
T = BC * K                   # tokens per core (8192)
TS = 1024                    # tokens per tile
NT = T // TS                 # tiles per core
GROUPS = TS // 128           # 128-token groups per tile
SPG = 128 // K               # samples per group (16)
LN_EPS = 1e-5
NEG = -30.0                  # large-negative mask for cross-sample scores

XS = 32.0                    # fp8 scale for x and attention output
WS = 4096.0                  # fp8 scale for wq/wv/wo
KS = 32768.0                 # fp8 scale for wk/sqrt(HD)
RS = float(2 ** 17)          # residual prescale (= XS*WS); LN absorbs it

F32 = mybir.dt.float32
BF16 = mybir.dt.bfloat16
FP8 = mybir.dt.float8e4
DR = mybir.MatmulPerfMode.DoubleRow

_CACHED = None  # compiled Bacc module, built once per process


def _build():
    nc = bacc.Bacc("TRN2", target_bir_lowering=False, debug=False, num_devices=1)

    xt8_d = nc.dram_tensor("XT8", [D, T], FP8, kind="ExternalInput").ap()
    xb_d = nc.dram_tensor("XB", [T, D], F32, kind="ExternalInput").ap()
    wq_d = nc.dram_tensor("WQ8", [D, D], FP8, kind="ExternalInput").ap()
    wk_d = nc.dram_tensor("WK8", [D, D], FP8, kind="ExternalInput").ap()
    wv_d = nc.dram_tensor("WV8", [D, D], FP8, kind="ExternalInput").ap()
    wo_d = nc.dram_tensor("WO8", [D, D], FP8, kind="ExternalInput").ap()
    bqk_d = nc.dram_tensor("BQK", [128, 2 * H], F32, kind="ExternalInput").ap()
    pm_d = nc.dram_tensor("PM", [128, 128], F32, kind="ExternalInput").ap()
    om_d = nc.dram_tensor("OM", [128, 128], BF16, kind="ExternalInput").ap()
    out_d = nc.dram_tensor("OUT", [T, D], F32, kind="ExternalOutput").ap()

    xt8_r = xt8_d.rearrange("(c p) t -> p c t", p=128)   # [128, 8, T]

    with tile.TileContext(nc) as tc:
        with (
            tc.tile_pool(name="wpool", bufs=1) as wpool,
            tc.tile_pool(name="consts", bufs=1) as consts,
            tc.tile_pool(name="xt8p", bufs=2) as xt8p,
            tc.tile_pool(name="qkp", bufs=2) as qkp,
            tc.tile_pool(name="vp", bufs=2) as vp,
            tc.tile_pool(name="ptp", bufs=2) as ptp,
            tc.tile_pool(name="rzbp", bufs=2) as rzbp,
            tc.tile_pool(name="ot8p", bufs=2) as ot8p,
            tc.tile_pool(name="xbp", bufs=GROUPS + 1) as xbp,
            tc.tile_pool(name="smalls", bufs=4) as smalls,
            # two independent 2-bank x 2-buf PSUM streams: projections
            # (psab/psv/yp) and attention (st/zb/ot) — 8 banks total, and the
            # next tile's QK never contends with this tile's attention chain
            tc.tile_pool(name="projps", bufs=2, space="PSUM") as projps,
            tc.tile_pool(name="attps", bufs=2, space="PSUM") as attps,
        ):
            # ---- constants / weights (resident) ----
            wq = wpool.tile([128, 8, D], FP8, tag="w_q")
            nc.sync.dma_start(wq[:], wq_d.rearrange("(c p) m -> p c m", p=128))
            wk = wpool.tile([128, 8, D], FP8, tag="w_k")
            nc.sync.dma_start(wk[:], wk_d.rearrange("(c p) m -> p c m", p=128))
            wv = wpool.tile([128, 8, D], FP8, tag="w_v")
            nc.sync.dma_start(wv[:], wv_d.rearrange("(c p) m -> p c m", p=128))
            wo = wpool.tile([128, 8, D], FP8, tag="w_o")
            nc.sync.dma_start(wo[:], wo_d.rearrange("(c p) m -> p c m", p=128))
            bqk = consts.tile([128, 2 * H], F32)
            nc.sync.dma_start(bqk[:], bqk_d)
            pm = consts.tile([128, 128], F32)
            nc.sync.dma_start(pm[:], pm_d)
            om = consts.tile([128, 128], BF16)
            nc.sync.dma_start(om[:], om_d)
            eps = consts.tile([128, 1], F32)
            nc.vector.memset(eps[:], LN_EPS * RS * RS)

            def start_tile(t):
                """Allocate tile t's SBUF tiles + input DMAs; return state."""
                tok0 = t * TS
                xt8 = xt8p.tile([128, 8, TS], FP8)
                nc.sync.dma_start(xt8[:], xt8_r[:, :, tok0 : tok0 + TS])
                s = {
                    "tok0": tok0,
                    "xt8": xt8,
                    "qt": qkp.tile([128, H, TS], BF16, tag="qt", name="qt"),
                    "kt": qkp.tile([128, H, TS], BF16, tag="kt", name="kt"),
                    "v": vp.tile([128, GROUPS, H, HD], BF16, tag="v", name="v"),
                    "mvt": smalls.tile([128, GROUPS, 2], F32, tag="mvt", name="mvt"),
                    "xbs": [],
                }
                for g in range(GROUPS):
                    xb = xbp.tile([128, D], F32)
                    nc.sync.dma_start(
                        xb[:], xb_d[tok0 + g * 128 : tok0 + (g + 1) * 128, :]
                    )
                    s["xbs"].append(xb)
                return s

            def emit_qk_unit(s, u):
                """One head of the Q or K projection (fp8 DoubleRow)."""
                proj, h = divmod(u, H)
                wt, dst, bias_col0, dsc = (
                    (wq, s["qt"], 0, 1.0 / RS),
                    (wk, s["kt"], H, 1.0 / (KS * XS)),
                )[proj]
                xt8 = s["xt8"]
                psab = projps.tile([128, 1024], F32, tag="projps")
                for c in range(4):
                    lw = wt[:, 2 * c : 2 * c + 2, h * HD : (h + 1) * HD]
                    nc.tensor.matmul(
                        psab[:, 0:512], lw, xt8[:, 2 * c : 2 * c + 2, 0:512],
                        start=(c == 0), stop=(c == 3), perf_mode=DR,
                    )
                    nc.tensor.matmul(
                        psab[:, 512:1024], lw, xt8[:, 2 * c : 2 * c + 2, 512:1024],
                        start=(c == 0), stop=(c == 3), perf_mode=DR,
                    )
                nc.scalar.activation(
                    dst[:, h, :], psab[:],
                    mybir.ActivationFunctionType.Identity,
                    bias=bqk[:, bias_col0 + h : bias_col0 + h + 1],
                    scale=dsc,
                )

            def emit_v_unit(s, sub):
                """One 128-token group of the V projection (fp8 DoubleRow)."""
                xt8 = s["xt8"]
                psv = projps.tile([128, 1024], F32, tag="projps")
                for c in range(4):
                    lx = xt8[:, 2 * c : 2 * c + 2, sub * 128 : (sub + 1) * 128]
                    nc.tensor.matmul(
                        psv[:, 0:512], lx, wv[:, 2 * c : 2 * c + 2, 0:512],
                        start=(c == 0), stop=(c == 3), perf_mode=DR,
                    )
                    nc.tensor.matmul(
                        psv[:, 512:1024], lx, wv[:, 2 * c : 2 * c + 2, 512:1024],
                        start=(c == 0), stop=(c == 3), perf_mode=DR,
                    )
                nc.scalar.activation(
                    s["v"][:, sub, :, :], psv.rearrange("p (a b) -> p a b", a=8),
                    mybir.ActivationFunctionType.Copy, scale=1.0 / RS,
                )

            def emit_unit(s, u):
                if u < 2 * H:
                    emit_qk_unit(s, u)
                else:
                    emit_v_unit(s, u - 2 * H)

            def emit_group(s, g, fillers=()):
                """Attention + output projection + residual + LN stats for one
                128-token (16-sample) group. `fillers` are next-tile projection
                units emitted at this group's two PE wait points (after the
                score matmuls while exp runs, and after ot while the
                reciprocal + fp8 quantize run) so the PE never idles."""
                qt, kt, v = s["qt"], s["kt"], s["v"]
                gsl = slice(g * 128, (g + 1) * 128)
                st = attps.tile([128, H, 128], F32, tag="attps")
                for h in range(H):
                    nc.tensor.matmul(st[:, h, :], kt[:, h, gsl], qt[:, h, gsl])
                # add prior/mask (same [128,128] table per head), in place
                nc.vector.tensor_tensor(
                    st[:], st[:],
                    pm[:, None, :].to_broadcast((128, H, 128)),
                    mybir.AluOpType.add,
                )
                pt = ptp.tile([128, H, 128], BF16)
                nc.scalar.activation(pt[:], st[:], mybir.ActivationFunctionType.Exp)
                for f in fillers[:2]:
                    f()
                # denominator, broadcast to all partitions: ZB = (1/32)^T P
                zb = attps.tile([128, 1024], F32, tag="attps")
                nc.tensor.matmul(zb[:, 0:512], om[:], pt[:, 0:4, :])
                nc.tensor.matmul(zb[:, 512:1024], om[:], pt[:, 4:8, :])
                rzb = rzbp.tile([128, H, 128], F32)
                nc.vector.reciprocal_approx_fast(
                    rzb[:], zb.rearrange("p (a b) -> p a b", a=8)
                )
                # OT[d, q] = V^T P per head (lhsT=v), then *32/Z -> fp8
                ot = attps.tile([128, H, 128], F32, tag="attps")
                for h in range(H):
                    nc.tensor.matmul(ot[:, h, :], v[:, g, h, :], pt[:, h, :])
                ot8 = ot8p.tile([128, H, 128], FP8)
                nc.vector.tensor_tensor(ot8[:], ot[:], rzb[:], mybir.AluOpType.mult)
                for f in fillers[2:]:
                    f()

                xb = s["xbs"][g]
                yp = attps.tile([128, 1024], F32, tag="attps")
                for c in range(4):
                    lo = ot8[:, 2 * c : 2 * c + 2, :]
                    nc.tensor.matmul(
                        yp[:, 0:512], lo, wo[:, 2 * c : 2 * c + 2, 0:512],
                        start=(c == 0), stop=(c == 3), perf_mode=DR,
                    )
                    nc.tensor.matmul(
                        yp[:, 512:1024], lo, wo[:, 2 * c : 2 * c + 2, 512:1024],
                        start=(c == 0), stop=(c == 3), perf_mode=DR,
                    )
                nc.vector.tensor_tensor(xb[:], xb[:], yp[:], mybir.AluOpType.add)
                stats = smalls.tile([128, 2, 6], F32, tag="stats")
                for sg in range(2):
                    nc.vector.bn_stats(
                        stats[:, sg, :], xb[:, sg * 512 : (sg + 1) * 512]
                    )
                nc.vector.bn_aggr(s["mvt"][:, g, :], stats[:])

            def emit_ln_finalize(s, last):
                """rstd = exp(-0.5*ln(var+eps')) batched: table switches happen
                per tile, not per group. The last tile splits in half to
                shorten the drain after the final matmul."""
                tok0, mvt = s["tok0"], s["mvt"]
                batches = ((0, 4), (4, 8)) if last else ((0, GROUPS),)
                sdt = smalls.tile([128, GROUPS], F32, tag="sdt")
                for lo, hi in batches:
                    nc.scalar.activation(
                        sdt[:, lo:hi], mvt[:, lo:hi, 1],
                        mybir.ActivationFunctionType.Ln, bias=eps[:],
                    )
                    nc.scalar.activation(
                        sdt[:, lo:hi], sdt[:, lo:hi],
                        mybir.ActivationFunctionType.Exp, scale=-0.5,
                    )
                    for g in range(lo, hi):
                        xb = s["xbs"][g]
                        nc.vector.tensor_scalar(
                            out=xb[:],
                            in0=xb[:],
                            scalar1=mvt[:, g, 0:1],
                            scalar2=sdt[:, g : g + 1],
                            op0=mybir.AluOpType.subtract,
                            op1=mybir.AluOpType.mult,
                        )
                        nc.sync.dma_start(
                            out_d[tok0 + g * 128 : tok0 + (g + 1) * 128, :], xb[:]
                        )

            # Software pipeline: tile t's 24 projection units (PE-dense, no
            # deps on tile t-1) are interleaved 3-per-group with tile t-1's
            # attention groups, so they fill the PE bubbles left by each
            # group's exp -> reciprocal -> quantize dependency chain.
            prev = None
            for t in range(NT):
                s = start_tile(t)
                if prev is None:
                    for u in range(3 * GROUPS):
                        emit_unit(s, u)
                else:
                    for g in range(GROUPS):
                        fillers = tuple(
                            (lambda u=u: emit_unit(s, u))
                            for u in range(3 * g, 3 * (g + 1))
                        )
                        emit_group(prev, g, fillers)
                    emit_ln_finalize(prev, last=False)
                prev = s
            for g in range(GROUPS):
                emit_group(prev, g)
            emit_ln_finalize(prev, last=True)

    nc.compile()
    return nc


def _get_nc():
    global _CACHED
    if _CACHED is None:
        _CACHED = _build()
    return _CACHED


def _reference_numpy(modality_encodings, selection_mask, wq, bq, wk, bk, wv, bv,
                     wo, bo, rel_prior, ln_gamma, ln_beta):
    """Slow fallback, exact port of the reference (used only if inputs fall
    outside the fast path's assumptions: non-trivial mask)."""
    x = modality_encodings.astype(np.float32)
    Bn, Kn, Dn = x.shape
    Hd = Dn // H
    q = (x @ wq.T + bq).reshape(Bn, Kn, H, Hd).transpose(0, 2, 1, 3)
    k = (x @ wk.T + bk).reshape(Bn, Kn, H, Hd).transpose(0, 2, 1, 3)
    v = (x @ wv.T + bv).reshape(Bn, Kn, H, Hd).transpose(0, 2, 1, 3)
    scores = np.einsum("bhqd,bhkd->bhqk", q, k) / math.sqrt(Hd)
    scores = scores + rel_prior[None, None]
    mask2d = (selection_mask[:, :, None] * selection_mask[:, None, :]) > 0
    scores = np.where(mask2d[:, None], scores, -np.inf)
    scores = scores - scores.max(axis=-1, keepdims=True)
    e = np.exp(scores)
    attn = e / e.sum(axis=-1, keepdims=True)
    out = np.einsum("bhqk,bhkd->bhqd", attn, v)
    out = out.transpose(0, 2, 1, 3).reshape(Bn, Kn, Dn)
    out = out @ wo.T + bo
    res = x + out
    mu = res.mean(-1, keepdims=True)
    var = ((res - mu) ** 2).mean(-1, keepdims=True)
    return (res - mu) / np.sqrt(var + LN_EPS) * ln_gamma + ln_beta


def _q8(a, scale):
    import ml_dtypes

    return np.clip(a * scale, -240.0, 240.0).astype(ml_dtypes.float8_e4m3)


def _prep_in_maps(modality_encodings, wq, bq, wk, bk, wv, bv, wo, bo, rel_prior):
    import ml_dtypes

    s = 1.0 / math.sqrt(HD)
    wq8 = _q8(np.ascontiguousarray(wq.T), WS)
    wk8 = _q8(np.ascontiguousarray((wk * s).T), KS)
    wv8 = _q8(np.ascontiguousarray(wv.T), WS)
    wo8 = _q8(np.ascontiguousarray(wo.T), WS)
    bks = bk * s
    b_eff = (bo + wo @ bv).astype(np.float32)

    bqk = np.concatenate(
        [bq.reshape(H, HD).T, bks.reshape(H, HD).T], axis=1
    ).astype(np.float32)  # [128, 16]

    pmat = np.full((128, 128), NEG, dtype=np.float32)
    for sm in range(SPG):
        pmat[sm * K : (sm + 1) * K, sm * K : (sm + 1) * K] = rel_prior.T
    omat = np.full((128, 128), 1.0 / XS, dtype=ml_dtypes.bfloat16)

    x_flat = modality_encodings.reshape(B * K, D)
    in_maps = []
    for c in range(N_CORES):
        x_core = x_flat[c * T : (c + 1) * T]
        in_maps.append({
            "XT8": _q8(np.ascontiguousarray(x_core.T), XS),
            "XB": (x_core + b_eff) * RS,
            "WQ8": wq8, "WK8": wk8, "WV8": wv8, "WO8": wo8,
            "BQK": bqk, "PM": pmat, "OM": omat,
        })
    return in_maps


def run_device(inputs, trace=False):
    """Build in_maps from full inputs, run on 8 cores, return (full_out, results)."""
    in_maps = _prep_in_maps(
        inputs["modality_encodings"], inputs["wq"], inputs["bq"], inputs["wk"],
        inputs["bk"], inputs["wv"], inputs["bv"], inputs["wo"], inputs["bo"],
        inputs["rel_prior"],
    )
    nc = _get_nc()
    res = run_bass_kernel_spmd(nc, in_maps, core_ids=list(range(N_CORES)), trace=trace)
    out = np.concatenate(
        [res.results[c]["OUT"].reshape(BC, K, D) for c in range(N_CORES)], axis=0
    )
    return out, res


def kernel(**inputs) -> np.ndarray:
    inputs = {k: np.asarray(v) for k, v in inputs.items()}
    mask = inputs["selection_mask"]
    gamma = inputs["ln_gamma"]
    beta = inputs["ln_beta"]
    if not np.all(mask > 0):
        # general-mask fallback (never hit for the spec'd inputs: fill=ones)
        return _reference_numpy(**{k: inputs[k].astype(np.float32) for k in (
            "modality_encodings", "selection_mask", "wq", "bq", "wk", "bk",
            "wv", "bv", "wo", "bo", "rel_prior", "ln_gamma", "ln_beta")}
        ).astype(np.float32)

    out, _ = run_device(inputs, trace=False)
    # device kernel skips the (identity for spec'd inputs) LN affine params
    if not (np.all(gamma == 1.0) and np.all(beta == 0.0)):
        out = out * gamma + beta
    return out.astype(np.float32)


# revision 27
# speedup vs baseline: 1.0841x; 1.0752x over previous
"""CrossModalityAttention Trainium2 kernel (v2: fp8 DoubleRow projections).

Full inputs -> full output; internally shards batch B=8192 across 8 NeuronCores
(pure data parallel). Per core: 1024 samples x K=8 modalities = 8192 tokens of
D=1024.

Device strategy (per core):
  - All four DxD projections run in fp8-e4m3 with perf_mode=DoubleRow (2
    fp8 MACs/cell/cycle, contraction chunk pairs packed in the AP's dim1).
    Power-of-2 quantization scales: x*32, wq/wv/wo*4096, (wk/sqrt(128))*32768,
    attention-output*32. Descales fold into the ACT bias stage (Q/K/V) or the
    2^17-prescaled residual XB (output proj), whose scale LayerNorm absorbs
    exactly via eps' = eps*2^34 (LN is scale-invariant).
  - Scores per 128-token group (16 samples x K=8) stay bf16:
    ST[(s,k),(s',q)] = Kh^T Qh via matmul(lhsT=kt, rhs=qt); prior/mask table
    adds rel_prior on the block diagonal and -30 off it; exp() -> bf16 probs.
  - Transpose-free tail: OT[d,q] = matmul(lhsT=V[k,d] bf16, rhs=P[k,q] bf16)
    gives the output projection's lhsT directly (no PE transposes). The
    softmax denominator reaches all 128 partitions via one rank-1 PE matmul
    ZB = (1/32 ones)^T @ P, then DVE reciprocal + multiply quantize OT to
    fp8 (the 1/32 folds the fp8 scale for free).
  - LayerNorm: rstd = exp(-0.5*ln(var+eps')) keeps every ACT function in one
    table set.
"""

import math

import numpy as np

import concourse.bacc as bacc
import concourse.bass as bass
import concourse.mybir as mybir
import concourse.tile as tile
from concourse.bass_utils import run_bass_kernel_spmd

N_CORES = 8
B, K, D = 8192, 8, 1024
H, HD = 8, 128
BC = B // N_CORES            # samples per core
T = BC * K                   # tokens per core (8192)
TS = 1024                    # tokens per tile
NT = T // TS                 # tiles per core
GROUPS = TS // 128           # 128-token groups per tile
SPG = 128 // K               # samples per group (16)
LN_EPS = 1e-5
NEG = -30.0                  # large-negative mask for cross-sample scores

XS = 32.0                    # fp8 scale for x and attention output
WS = 4096.0                  # fp8 scale for wq/wv/wo
KS = 32768.0                 # fp8 scale for wk/sqrt(HD)
RS = float(2 ** 17)          # residual prescale (= XS*WS); LN absorbs it

F32 = mybir.dt.float32
BF16 = mybir.dt.bfloat16
FP8 = mybir.dt.float8e4
DR = mybir.MatmulPerfMode.DoubleRow

_CACHED = None  # compiled Bacc module, built once per process


def _build():
    nc = bacc.Bacc("TRN2", target_bir_lowering=False, debug=False, num_devices=1)

    xt8_d = nc.dram_tensor("XT8", [D, T], FP8, kind="ExternalInput").ap()
    xb_d = nc.dram_tensor("XB", [T, D], F32, kind="ExternalInput").ap()
    wq_d = nc.dram_tensor("WQ8", [D, D], FP8, kind="ExternalInput").ap()
    wk_d = nc.dram_tensor("WK8", [D, D], FP8, kind="ExternalInput").ap()
    wv_d = nc.dram_tensor("WV8", [D, D], FP8, kind="ExternalInput").ap()
    wo_d = nc.dram_tensor("WO8", [D, D], FP8, kind="ExternalInput").ap()
    bqk_d = nc.dram_tensor("BQK", [128, 2 * H], F32, kind="ExternalInput").ap()
    pm_d = nc.dram_tensor("PM", [128, 128], F32, kind="ExternalInput").ap()
    om_d = nc.dram_tensor("OM", [128, 128], BF16, kind="ExternalInput").ap()
    out_d = nc.dram_tensor("OUT", [T, D], F32, kind="ExternalOutput").ap()

    xt8_r = xt8_d.rearrange("(c p) t -> p c t", p=128)   # [128, 8, T]

    with tile.TileContext(nc) as tc:
        with (
            tc.tile_pool(name="wpool", bufs=1) as wpool,
            tc.tile_pool(name="consts", bufs=1) as consts,
            tc.tile_pool(name="xt8p", bufs=2) as xt8p,
            tc.tile_pool(name="qkp", bufs=2) as qkp,
            tc.tile_pool(name="vp", bufs=2) as vp,
            tc.tile_pool(name="ptp", bufs=GROUPS + 1) as ptp,
            tc.tile_pool(name="rzbp", bufs=2) as rzbp,
            tc.tile_pool(name="ot8p", bufs=2) as ot8p,
            tc.tile_pool(name="xbp", bufs=GROUPS) as xbp,
            tc.tile_pool(name="smalls", bufs=4) as smalls,
            # two independent 2-bank x 2-buf PSUM streams: projections
            # (psab/psv/yp) and attention (st/zb/ot) — 8 banks total, and the
            # next tile's QK never contends with this tile's attention chain
            tc.tile_pool(name="projps", bufs=2, space="PSUM") as projps,
            tc.tile_pool(name="attps", bufs=2, space="PSUM") as attps,
        ):
            # ---- constants / weights (resident) ----
            wq = wpool.tile([128, 8, D], FP8, tag="w_q")
            nc.sync.dma_start(wq[:], wq_d.rearrange("(c p) m -> p c m", p=128))
            wk = wpool.tile([128, 8, D], FP8, tag="w_k")
            nc.sync.dma_start(wk[:], wk_d.rearrange("(c p) m -> p c m", p=128))
            wv = wpool.tile([128, 8, D], FP8, tag="w_v")
            nc.sync.dma_start(wv[:], wv_d.rearrange("(c p) m -> p c m", p=128))
            wo = wpool.tile([128, 8, D], FP8, tag="w_o")
            nc.sync.dma_start(wo[:], wo_d.rearrange("(c p) m -> p c m", p=128))
            bqk = consts.tile([128, 2 * H], F32)
            nc.sync.dma_start(bqk[:], bqk_d)
            pm = consts.tile([128, 128], F32)
            nc.sync.dma_start(pm[:], pm_d)
            om = consts.tile([128, 128], BF16)
            nc.sync.dma_start(om[:], om_d)
            eps = consts.tile([128, 1], F32)
            nc.vector.memset(eps[:], LN_EPS * RS * RS)

            def start_tile(t):
                """Allocate tile t's SBUF tiles + input DMAs; return state."""
                tok0 = t * TS
                xt8 = xt8p.tile([128, 8, TS], FP8)
                nc.sync.dma_start(xt8[:], xt8_r[:, :, tok0 : tok0 + TS])
                s = {
                    "tok0": tok0,
                    "xt8": xt8,
                    "qt": qkp.tile([128, H, TS], BF16, tag="qt", name="qt"),
                    "kt": qkp.tile([128, H, TS], BF16, tag="kt", name="kt"),
                    "v": vp.tile([128, GROUPS, H, HD], BF16, tag="v", name="v"),
                    "mvt": smalls.tile([128, GROUPS, 2], F32, tag="mvt", name="mvt"),
                    "xbs": [],
                    "pts": [],
                }
                for g in range(GROUPS):
                    xb = xbp.tile([128, D], F32)
                    nc.sync.dma_start(
                        xb[:], xb_d[tok0 + g * 128 : tok0 + (g + 1) * 128, :]
                    )
                    s["xbs"].append(xb)
                return s

            def emit_qk_unit(s, u):
                """One head of the Q or K projection (fp8 DoubleRow)."""
                proj, h = divmod(u, H)
                wt, dst, bias_col0, dsc = (
                    (wq, s["qt"], 0, 1.0 / RS),
                    (wk, s["kt"], H, 1.0 / (KS * XS)),
                )[proj]
                xt8 = s["xt8"]
                psab = projps.tile([128, 1024], F32, tag="projps")
                for c in range(4):
                    lw = wt[:, 2 * c : 2 * c + 2, h * HD : (h + 1) * HD]
                    nc.tensor.matmul(
                        psab[:, 0:512], lw, xt8[:, 2 * c : 2 * c + 2, 0:512],
                        start=(c == 0), stop=(c == 3), perf_mode=DR,
                    )
                    nc.tensor.matmul(
                        psab[:, 512:1024], lw, xt8[:, 2 * c : 2 * c + 2, 512:1024],
                        start=(c == 0), stop=(c == 3), perf_mode=DR,
                    )
                nc.scalar.activation(
                    dst[:, h, :], psab[:],
                    mybir.ActivationFunctionType.Identity,
                    bias=bqk[:, bias_col0 + h : bias_col0 + h + 1],
                    scale=dsc,
                )

            def emit_v_unit(s, sub):
                """One 128-token group of the V projection (fp8 DoubleRow)."""
                xt8 = s["xt8"]
                psv = projps.tile([128, 1024], F32, tag="projps")
                for c in range(4):
                    lx = xt8[:, 2 * c : 2 * c + 2, sub * 128 : (sub + 1) * 128]
                    nc.tensor.matmul(
                        psv[:, 0:512], lx, wv[:, 2 * c : 2 * c + 2, 0:512],
                        start=(c == 0), stop=(c == 3), perf_mode=DR,
                    )
                    nc.tensor.matmul(
                        psv[:, 512:1024], lx, wv[:, 2 * c : 2 * c + 2, 512:1024],
                        start=(c == 0), stop=(c == 3), perf_mode=DR,
                    )
                nc.scalar.activation(
                    s["v"][:, sub, :, :], psv.rearrange("p (a b) -> p a b", a=8),
                    mybir.ActivationFunctionType.Copy, scale=1.0 / RS,
                )

            def emit_unit(s, u):
                if u < 2 * H:
                    emit_qk_unit(s, u)
                else:
                    emit_v_unit(s, u - 2 * H)

            def emit_stexp(s, g):
                """Phase A of a group: scores + prior/mask + exp -> pt (SBUF).
                Hoisted ahead of phase B so exp latency never blocks the PE."""
                qt, kt = s["qt"], s["kt"]
                gsl = slice(g * 128, (g + 1) * 128)
                st = attps.tile([128, H, 128], F32, tag="attps")
                for h in range(H):
                    nc.tensor.matmul(st[:, h, :], kt[:, h, gsl], qt[:, h, gsl])
                # add prior/mask (same [128,128] table per head), in place
                nc.vector.tensor_tensor(
                    st[:], st[:],
                    pm[:, None, :].to_broadcast((128, H, 128)),
                    mybir.AluOpType.add,
                )
                pt = ptp.tile([128, H, 128], BF16)
                nc.scalar.activation(pt[:], st[:], mybir.ActivationFunctionType.Exp)
                s["pts"].append(pt)

            def emit_groupB(s, g):
                """Phase B: denominator, O^T = V^T P, fp8 quantize, output
                projection, residual add and LN stats for one group."""
                v, pt = s["v"], s["pts"][g]
                # denominator, broadcast to all partitions: ZB = (1/32)^T P
                zb = attps.tile([128, 1024], F32, tag="attps")
                nc.tensor.matmul(zb[:, 0:512], om[:], pt[:, 0:4, :])
                nc.tensor.matmul(zb[:, 512:1024], om[:], pt[:, 4:8, :])
                rzb = rzbp.tile([128, H, 128], F32)
                nc.vector.reciprocal_approx_fast(
                    rzb[:], zb.rearrange("p (a b) -> p a b", a=8)
                )
                # OT[d, q] = V^T P per head (lhsT=v), then *32/Z -> fp8
                ot = attps.tile([128, H, 128], F32, tag="attps")
                for h in range(H):
                    nc.tensor.matmul(ot[:, h, :], v[:, g, h, :], pt[:, h, :])
                ot8 = ot8p.tile([128, H, 128], FP8)
                nc.vector.tensor_tensor(ot8[:], ot[:], rzb[:], mybir.AluOpType.mult)

                xb = s["xbs"][g]
                yp = attps.tile([128, 1024], F32, tag="attps")
                for c in range(4):
                    lo = ot8[:, 2 * c : 2 * c + 2, :]
                    nc.tensor.matmul(
                        yp[:, 0:512], lo, wo[:, 2 * c : 2 * c + 2, 0:512],
                        start=(c == 0), stop=(c == 3), perf_mode=DR,
                    )
                    nc.tensor.matmul(
                        yp[:, 512:1024], lo, wo[:, 2 * c : 2 * c + 2, 512:1024],
                        start=(c == 0), stop=(c == 3), perf_mode=DR,
                    )
                nc.vector.tensor_tensor(xb[:], xb[:], yp[:], mybir.AluOpType.add)
                stats = smalls.tile([128, 2, 6], F32, tag="stats")
                for sg in range(2):
                    nc.vector.bn_stats(
                        stats[:, sg, :], xb[:, sg * 512 : (sg + 1) * 512]
                    )
                nc.vector.bn_aggr(s["mvt"][:, g, :], stats[:])

            def emit_ln_finalize(s, last):
                """rstd = exp(-0.5*ln(var+eps')) batched: table switches happen
                per tile, not per group. The last tile splits in half to
                shorten the drain after the final matmul."""
                tok0, mvt = s["tok0"], s["mvt"]
                batches = ((0, 4), (4, 8)) if last else ((0, GROUPS),)
                sdt = smalls.tile([128, GROUPS], F32, tag="sdt")
                for lo, hi in batches:
                    nc.scalar.activation(
                        sdt[:, lo:hi], mvt[:, lo:hi, 1],
                        mybir.ActivationFunctionType.Ln, bias=eps[:],
                    )
                    nc.scalar.activation(
                        sdt[:, lo:hi], sdt[:, lo:hi],
                        mybir.ActivationFunctionType.Exp, scale=-0.5,
                    )
                    for g in range(lo, hi):
                        xb = s["xbs"][g]
                        nc.vector.tensor_scalar(
                            out=xb[:],
                            in0=xb[:],
                            scalar1=mvt[:, g, 0:1],
                            scalar2=sdt[:, g : g + 1],
                            op0=mybir.AluOpType.subtract,
                            op1=mybir.AluOpType.mult,
                        )
                        nc.sync.dma_start(
                            out_d[tok0 + g * 128 : tok0 + (g + 1) * 128, :], xb[:]
                        )

            # Software pipeline: tile t's 24 projection units (PE-dense, no
            # deps on tile t-1) interleave with tile t-1's attention groups.
            # Phase A hoists all scores+exp so pt is always ready; phase B's
            # reciprocal/quantize waits are covered by the next group's
            # independent matmuls and the interleaved units.
            prev = None
            for t in range(NT):
                s = start_tile(t)
                if prev is None:
                    for u in range(3 * GROUPS):
                        emit_unit(s, u)
                else:
                    for g in range(GROUPS):
                        emit_stexp(prev, g)
                        emit_unit(s, g)
                    for g in range(GROUPS):
                        emit_unit(s, GROUPS + 2 * g)
                        emit_groupB(prev, g)
                        emit_unit(s, GROUPS + 2 * g + 1)
                    emit_ln_finalize(prev, last=False)
                prev = s
            for g in range(GROUPS):
                emit_stexp(prev, g)
            for g in range(GROUPS):
                emit_groupB(prev, g)
            emit_ln_finalize(prev, last=True)

    nc.compile()
    return nc


def _get_nc():
    global _CACHED
    if _CACHED is None:
        _CACHED = _build()
    return _CACHED


def _reference_numpy(modality_encodings, selection_mask, wq, bq, wk, bk, wv, bv,
                     wo, bo, rel_prior, ln_gamma, ln_beta):
    """Slow fallback, exact port of the reference (used only if inputs fall
    outside the fast path's assumptions: non-trivial mask)."""
    x = modality_encodings.astype(np.float32)
    Bn, Kn, Dn = x.shape
    Hd = Dn // H
    q = (x @ wq.T + bq).reshape(Bn, Kn, H, Hd).transpose(0, 2, 1, 3)
    k = (x @ wk.T + bk).reshape(Bn, Kn, H, Hd).transpose(0, 2, 1, 3)
    v = (x @ wv.T + bv).reshape(Bn, Kn, H, Hd).transpose(0, 2, 1, 3)
    scores = np.einsum("bhqd,bhkd->bhqk", q, k) / math.sqrt(Hd)
    scores = scores + rel_prior[None, None]
    mask2d = (selection_mask[:, :, None] * selection_mask[:, None, :]) > 0
    scores = np.where(mask2d[:, None], scores, -np.inf)
    scores = scores - scores.max(axis=-1, keepdims=True)
    e = np.exp(scores)
    attn = e / e.sum(axis=-1, keepdims=True)
    out = np.einsum("bhqk,bhkd->bhqd", attn, v)
    out = out.transpose(0, 2, 1, 3).reshape(Bn, Kn, Dn)
    out = out @ wo.T + bo
    res = x + out
    mu = res.mean(-1, keepdims=True)
    var = ((res - mu) ** 2).mean(-1, keepdims=True)
    return (res - mu) / np.sqrt(var + LN_EPS) * ln_gamma + ln_beta


def _q8(a, scale):
    import ml_dtypes

    return np.clip(a * scale, -240.0, 240.0).astype(ml_dtypes.float8_e4m3)


def _prep_in_maps(modality_encodings, wq, bq, wk, bk, wv, bv, wo, bo, rel_prior):
    import ml_dtypes

    s = 1.0 / math.sqrt(HD)
    wq8 = _q8(np.ascontiguousarray(wq.T), WS)
    wk8 = _q8(np.ascontiguousarray((wk * s).T), KS)
    wv8 = _q8(np.ascontiguousarray(wv.T), WS)
    wo8 = _q8(np.ascontiguousarray(wo.T), WS)
    bks = bk * s
    b_eff = (bo + wo @ bv).astype(np.float32)

    bqk = np.concatenate(
        [bq.reshape(H, HD).T, bks.reshape(H, HD).T], axis=1
    ).astype(np.float32)  # [128, 16]

    pmat = np.full((128, 128), NEG, dtype=np.float32)
    for sm in range(SPG):
        pmat[sm * K : (sm + 1) * K, sm * K : (sm + 1) * K] = rel_prior.T
    omat = np.full((128, 128), 1.0 / XS, dtype=ml_dtypes.bfloat16)

    x_flat = modality_encodings.reshape(B * K, D)
    in_maps = []
    for c in range(N_CORES):
        x_core = x_flat[c * T : (c + 1) * T]
        in_maps.append({
            "XT8": _q8(np.ascontiguousarray(x_core.T), XS),
            "XB": (x_core + b_eff) * RS,
            "WQ8": wq8, "WK8": wk8, "WV8": wv8, "WO8": wo8,
            "BQK": bqk, "PM": pmat, "OM": omat,
        })
    return in_maps


def run_device(inputs, trace=False):
    """Build in_maps from full inputs, run on 8 cores, return (full_out, results)."""
    in_maps = _prep_in_maps(
        inputs["modality_encodings"], inputs["wq"], inputs["bq"], inputs["wk"],
        inputs["bk"], inputs["wv"], inputs["bv"], inputs["wo"], inputs["bo"],
        inputs["rel_prior"],
    )
    nc = _get_nc()
    res = run_bass_kernel_spmd(nc, in_maps, core_ids=list(range(N_CORES)), trace=trace)
    out = np.concatenate(
        [res.results[c]["OUT"].reshape(BC, K, D) for c in range(N_CORES)], axis=0
    )
    return out, res


def kernel(**inputs) -> np.ndarray:
    inputs = {k: np.asarray(v) for k, v in inputs.items()}
    mask = inputs["selection_mask"]
    gamma = inputs["ln_gamma"]
    beta = inputs["ln_beta"]
    if not np.all(mask > 0):
        # general-mask fallback (never hit for the spec'd inputs: fill=ones)
        return _reference_numpy(**{k: inputs[k].astype(np.float32) for k in (
            "modality_encodings", "selection_mask", "wq", "bq", "wk", "bk",
            "wv", "bv", "wo", "bo", "rel_prior", "ln_gamma", "ln_beta")}
        ).astype(np.float32)

    out, _ = run_device(inputs, trace=False)
    # device kernel skips the (identity for spec'd inputs) LN affine params
    if not (np.all(gamma == 1.0) and np.all(beta == 0.0)):
        out = out * gamma + beta
    return out.astype(np.float32)
